# revision 1
# baseline (speedup 1.0000x reference)
"""DAG-SCM Trainium2 kernel.

Computes the reference nn_DAGSCM model: a 128-node topological scan
(x_i = relu(w.x_parents + b) + sigma_i * z_i) over n_samples, with the
per-node noise scale sigma_i calibrated from a tiny pilot pass
(0.1 * IQR, computed on host - it is a [128, 256] problem).

Strategy (memory-bound target):
  - Data-parallel over 8 NeuronCores on the sample axis.
  - Per core, samples live as [128 partitions x F free] tiles; each DAG
    node is one free-dim slice. The DAG structure and all per-node
    scalars (w0, w1, b, sigma) are baked into the traced Bass program
    as immediates / AP offsets at runtime (the kernel is JIT-traced
    with the actual inputs in hand).
  - Per non-root node:
        u  = w1 * p1 + b        (ScalarE activation, Identity)
        s  = w0 * p0 + u        (DVE scalar_tensor_tensor)
        zs = sigma * z          (GPSIMD tensor_scalar)
        v  = max(s, 0) + zs     (DVE/GPSIMD stt - fused relu+noise)
    Nodes in the output set write v directly into an interleaved output
    tile [p, f*64 + j] so the final DMA to the [n_samples, 64] output is
    fully contiguous per partition.
  - Only ancestors of the chosen output nodes are computed; only their
    z_noise rows are loaded (100 of 128 rows for this DAG).
"""

import numpy as np

N_CORES = 8
P = 128  # SBUF partitions
CAL_FRAC = 0.1


def _host_pilot_sigma(W_eff, b, parents, is_root, root_pilot):
    """Noiseless pilot scan + per-node sigma = CAL_FRAC * IQR (host, f32)."""
    n_nodes = len(parents)
    n = root_pilot.shape[1]
    vals = np.zeros((n_nodes, n), np.float32)
    for i in range(n_nodes):
        if is_root[i]:
            v = root_pilot[i].astype(np.float32)
        else:
            h = np.zeros(n, np.float32)
            for p, w in parents[i]:
                h = h + np.float32(w) * vals[p]
            v = np.maximum(h + np.float32(b[i]), np.float32(0.0))
        v = np.where(np.isfinite(v), v, np.float32(0.0))
        vals[i] = v
    q75 = np.quantile(vals.astype(np.float64), 0.75, axis=1)
    q25 = np.quantile(vals.astype(np.float64), 0.25, axis=1)
    sigma = CAL_FRAC * np.maximum(q75 - q25, 1e-6)
    return sigma.astype(np.float32)


def _dag_structure(W, b, par_idx, par_mask, is_root, chosen):
    n_nodes = W.shape[0]
    W_eff = (np.asarray(W, np.float32) * np.asarray(par_mask, np.float32))
    parents = []
    for i in range(n_nodes):
        ps = [
            (int(par_idx[i, j]), float(W_eff[i, j]))
            for j in range(par_idx.shape[1])
            if par_mask[i, j] > 0
        ]
        parents.append(ps)
    # needed = chosen + all ancestors
    needed = set(int(c) for c in chosen)
    for i in range(n_nodes - 1, -1, -1):
        if i in needed and not is_root[i]:
            for p, _ in parents[i]:
                needed.add(p)
    return W_eff, parents, needed


def _build_program(NLOC, parents, is_root, chosen, needed, b, sigma, n_nodes,
                   op3_pool_mod=0, gz=17, repeats=1, plan="slack",
                   zs_pool_frac=0.6, slack_thresh=2, dt16=True,
                   host_zscale=True):
    """Trace the per-core Bass/Tile program. Returns (nc, z_rows, root_rows)."""
    from concourse import bacc
    import concourse.mybir as mybir
    from concourse.tile import TileContext

    F = NLOC // P
    assert NLOC % P == 0

    f32 = mybir.dt.float32
    cdt = mybir.dt.float16 if dt16 else f32
    AF = mybir.ActivationFunctionType
    OP = mybir.AluOpType

    # columns of the output each node must write (normally 0 or 1)
    node_cols = {}
    for j, c in enumerate(int(c) for c in chosen):
        node_cols.setdefault(c, []).append(j)
    n_out = len(chosen)

    topo = [i for i in range(n_nodes) if i in needed]  # index order == topo
    topo_pos = {node: k for k, node in enumerate(topo)}
    z_rows = [i for i in topo if not is_root[i]]
    z_row_of = {node: r for r, node in enumerate(z_rows)}
    root_rows = [i for i in topo if is_root[i]]
    root_row_of = {node: r for r, node in enumerate(root_rows)}
    # nodes that need a contiguous vals slice: non-chosen needed nodes,
    # plus all roots (DMA target must be contiguous)
    vals_nodes = [i for i in topo if i not in node_cols or is_root[i]]
    n_z = len(z_rows)

    nc = bacc.Bacc(None, target_bir_lowering=False)
    z_in = nc.dram_tensor("zin", [max(n_z, 1), NLOC], cdt,
                          kind="ExternalInput")
    root_in = nc.dram_tensor("rootin", [max(len(root_rows), 1), NLOC], f32,
                             kind="ExternalInput")
    out_d = nc.dram_tensor("out", [NLOC, n_out], f32, kind="ExternalOutput")

    with TileContext(nc) as tc:
        with tc.tile_pool(name="vals", bufs=1) as vpool, \
             tc.tile_pool(name="zpool", bufs=4) as zpool, \
             tc.tile_pool(name="tmp", bufs=24) as tpool, \
             tc.tile_pool(name="outp", bufs=1) as opool:

            out_t = opool.tile([P, n_out * F], f32, tag="out", name="out_t")
            out_cols = out_t[:].rearrange("p (f j) -> p j f", j=n_out)

            vtile = {}
            for i in vals_nodes:
                vtile[i] = vpool.tile([P, F], cdt, tag=f"v{i}", name=f"vt{i}")

            def col_ap(j):
                return out_cols[:, j]

            def src_ap(node):
                if node in vtile:
                    return vtile[node][:]
                return col_ap(node_cols[node][0])

            def dst_aps(node):
                if node in node_cols and not is_root[node]:
                    return [col_ap(j) for j in node_cols[node]]
                return [vtile[node][:]]

            def trace_body(rep):
                # root rows: DMA to f32 staging, then into vals slices
                for r in root_rows:
                    rs = tpool.tile([P, F], f32, tag="rootstage",
                                    name=f"rs{rep}_{r}")
                    nc.sync.dma_start(
                        out=rs[:],
                        in_=root_in[root_row_of[r]:root_row_of[r] + 1, :]
                            .rearrange("o (p f) -> (o p) f", p=P),
                    )
                    nc.vector.tensor_copy(out=vtile[r][:], in_=rs[:])
                    for j in node_cols.get(r, []):
                        nc.scalar.copy(out=col_ap(j), in_=rs[:])

                # z row groups (node-order); DMA traced at group boundaries
                z_group_tiles = {}

                def ensure_z_group(g):
                    if g in z_group_tiles:
                        return z_group_tiles[g]
                    r0 = g * gz
                    r1 = min(r0 + gz, n_z)
                    zt = zpool.tile([P, (r1 - r0) * F], cdt, tag="zg",
                                    name=f"zg{rep}_{g}")
                    nc.sync.dma_start(
                        out=zt[:].rearrange("p (r f) -> p r f", r=r1 - r0),
                        in_=z_in[r0:r1, :].rearrange("r (p f) -> p r f", p=P),
                    )
                    z_group_tiles[g] = zt
                    return zt

                def z_ap(node):
                    r = z_row_of[node]
                    g, k = divmod(r, gz)
                    zt = ensure_z_group(g)
                    return zt[:, k * F:(k + 1) * F]

                n_nonroot = len(z_rows)
                for i in topo:
                    if is_root[i]:
                        continue
                    ps = parents[i]
                    bi = float(b[i])
                    si = float(sigma[i])
                    dsts = dst_aps(i)
                    if len(ps) == 0:
                        # v = relu(b) + sigma*z in one op
                        c = max(bi, 0.0)
                        s1 = 1.0 if host_zscale else si
                        nc.vector.tensor_scalar(
                            out=dsts[0], in0=z_ap(i),
                            scalar1=s1, scalar2=c, op0=OP.mult, op1=OP.add)
                    else:
                        # noise prescale zs = sigma * z (slack dep: only
                        # needs the z DMA, so off-chain engines are fine).
                        # With host_zscale, sigma is folded into the input
                        # and op3 reads the z tile directly.
                        if host_zscale:
                            zs_src = z_ap(i)
                        else:
                            zs_t = tpool.tile([P, F], cdt, tag="zs",
                                              name=f"zs{rep}_{i}")
                            frac = z_row_of[i] / max(n_nonroot - 1, 1)
                            if plan == "base":
                                zs_eng = "pool"
                            else:
                                zs_eng = ("pool" if frac < zs_pool_frac
                                          else "act")
                            if zs_eng == "pool":
                                nc.gpsimd.tensor_scalar(
                                    out=zs_t[:], in0=z_ap(i),
                                    scalar1=si, scalar2=None, op0=OP.mult)
                            else:
                                nc.scalar.activation(
                                    zs_t[:], z_ap(i), AF.Copy,
                                    bias=0.0, scale=si)
                            zs_src = zs_t[:]
                        if len(ps) >= 2:
                            # pick the parent with more scheduling slack
                            # for op1 (off-chain candidate)
                            pa, pb = ps[1], ps[0]
                            if plan == "slack":
                                pa, pb = sorted(
                                    ps, key=lambda pw: topo_pos[pw[0]])[0:2]
                            u_t = tpool.tile([P, F], cdt, tag="u",
                                             name=f"u{rep}_{i}")
                            op1_act = (
                                plan == "base"
                                or (plan == "slack"
                                    and topo_pos[i] - topo_pos[pa[0]]
                                    >= slack_thresh and bi == 0.0))
                            if op1_act and bi == 0.0:
                                nc.scalar.activation(
                                    u_t[:], src_ap(pa[0]), AF.Identity,
                                    bias=bi, scale=pa[1])
                            else:
                                nc.vector.tensor_scalar(
                                    out=u_t[:], in0=src_ap(pa[0]),
                                    scalar1=pa[1], scalar2=bi,
                                    op0=OP.mult, op1=OP.add)
                            s_t = tpool.tile([P, F], cdt, tag="s",
                                             name=f"s{rep}_{i}")
                            nc.vector.scalar_tensor_tensor(
                                out=s_t[:], in0=src_ap(pb[0]),
                                scalar=pb[1], in1=u_t[:],
                                op0=OP.mult, op1=OP.add)
                        else:
                            s_t = tpool.tile([P, F], cdt, tag="s",
                                             name=f"s{rep}_{i}")
                            nc.vector.tensor_scalar(
                                out=s_t[:], in0=src_ap(ps[0][0]),
                                scalar1=ps[0][1], scalar2=bi,
                                op0=OP.mult, op1=OP.add)
                        # v = max(s, 0) + zs  (fused relu + noise add)
                        nc.vector.scalar_tensor_tensor(
                            out=dsts[0], in0=s_t[:], scalar=0.0, in1=zs_src,
                            op0=OP.max, op1=OP.add)
                    for extra in dsts[1:]:
                        nc.scalar.copy(out=extra, in_=dsts[0])

                # output DMA: all 128 partitions per transfer, split along
                # the free dim across several dma_starts
                out_ap = out_d[:, :].rearrange("(p f) j -> p (f j)", p=P)
                FSPLIT = (F + 3) // 4
                for f0 in range(0, F, FSPLIT):
                    f1 = min(f0 + FSPLIT, F)
                    nc.sync.dma_start(
                        out=out_ap[:, f0 * n_out:f1 * n_out],
                        in_=out_t[:, f0 * n_out:f1 * n_out])

            for rep in range(repeats):
                trace_body(rep)

    nc.finalize()
    return nc, z_rows, root_rows


_CACHE = {}
_LAST_NC = None
_LAST_IN_MAPS = None


def _get_program(key, *args, **kwargs):
    if key not in _CACHE:
        _CACHE[key] = _build_program(*args, **kwargs)
    return _CACHE[key]


def run(n_samples, W, b, root_pilot, root_main, z_noise, par_mask, par_idx,
        is_root, chosen, trace=False, n_cores=N_CORES, op3_pool_mod=0, gz=17,
        repeats=1, plan="slack", zs_pool_frac=0.6, slack_thresh=2,
        dt16=True, host_zscale=True):
    W = np.asarray(W, np.float32)
    b = np.asarray(b, np.float32)
    root_pilot = np.asarray(root_pilot, np.float32)
    root_main = np.asarray(root_main, np.float32)
    z_noise = np.asarray(z_noise, np.float32)
    par_mask = np.asarray(par_mask, np.float32)
    par_idx = np.asarray(par_idx, np.int32)
    is_root = np.asarray(is_root, bool)
    chosen = np.asarray(chosen, np.int32)

    n_nodes = W.shape[0]
    NS = root_main.shape[1]
    assert NS % (n_cores * P) == 0
    NLOC = NS // n_cores

    W_eff, parents, needed = _dag_structure(W, b, par_idx, par_mask, is_root,
                                            chosen)
    sigma = _host_pilot_sigma(W_eff, b, parents, is_root, root_pilot)

    key = (NLOC, n_nodes, tuple(chosen.tolist()), par_idx.tobytes(),
           par_mask.tobytes(), W_eff.tobytes(), b.tobytes(), sigma.tobytes(),
           is_root.tobytes(), op3_pool_mod, gz, repeats, plan,
           zs_pool_frac, slack_thresh, dt16, host_zscale)
    nc, z_rows, root_rows = _get_program(
        key, NLOC, parents, is_root, chosen, needed, b, sigma, n_nodes,
        op3_pool_mod=op3_pool_mod, gz=gz, repeats=repeats, plan=plan,
        zs_pool_frac=zs_pool_frac, slack_thresh=slack_thresh, dt16=dt16,
        host_zscale=host_zscale)

    zdt = np.float16 if dt16 else np.float32
    if z_rows:
        zsel = z_noise[z_rows]
        if host_zscale:
            zsel = zsel * sigma[z_rows][:, None]
        z_packed = np.ascontiguousarray(zsel.astype(zdt))
    else:
        z_packed = np.zeros((1, NS), zdt)
    root_packed = np.ascontiguousarray(root_main[root_rows]) if root_rows \
        else np.zeros((1, NS), np.float32)

    in_maps = []
    for c in range(n_cores):
        s0, s1 = c * NLOC, (c + 1) * NLOC
        in_maps.append({
            "zin": np.ascontiguousarray(z_packed[:, s0:s1]),
            "rootin": np.ascontiguousarray(root_packed[:, s0:s1]),
        })

    from concourse.bass_utils import run_bass_kernel_spmd
    global _LAST_NC, _LAST_IN_MAPS
    _LAST_NC, _LAST_IN_MAPS = nc, in_maps
    res = run_bass_kernel_spmd(nc, in_maps, core_ids=list(range(n_cores)),
                               trace=trace)
    out = np.concatenate([np.asarray(r["out"]) for r in res.results], axis=0)
    return out.astype(np.float32, copy=False), res


def kernel(**inputs):
    out, _ = run(**inputs)
    return out



# revision 2
# speedup vs baseline: 2.7223x; 2.7223x over previous
"""DAG-SCM Trainium2 kernel.

Computes the reference nn_DAGSCM model: a 128-node topological scan
(x_i = relu(w.x_parents + b) + sigma_i * z_i) over n_samples, with the
per-node noise scale sigma_i calibrated from a tiny pilot pass
(0.1 * IQR, computed on host - it is a [128, 256] problem).

Strategy (memory-bound target):
  - Data-parallel over 8 NeuronCores on the sample axis.
  - Per core, samples live as [128 partitions x F free] tiles; each DAG
    node is one free-dim slice. The DAG structure and all per-node
    scalars (w0, w1, b, sigma) are baked into the traced Bass program
    as immediates / AP offsets at runtime (the kernel is JIT-traced
    with the actual inputs in hand).
  - Host<->device traffic is the dominant cost, so it is minimized:
      * noise rows with sigma < 1e-4 are dropped entirely (their
        contribution is < ~5e-4 absolute, far below the error budget);
        for this DAG that cuts ~100 noise rows down to ~39.
      * the kept noise rows are pre-scaled by sigma on host and
        uploaded as fp8 e4m3 in a partition-major layout; the device
        casts fp8 -> fp16 during the load DMA (SWDGE).
      * the root row is uploaded as fp16.
      * the output DRAM tensor is fp16 (upcast to f32 on host).
  - Per non-root noisy node:
        u  = w1 * p1 + b        (TS / ACT activation Identity)
        s  = w0 * p0 + u        (DVE scalar_tensor_tensor)
        v  = max(s, 0) + zs     (DVE stt - fused relu+noise)
    Per quiet node (noise dropped): the last op is a relu only
    (DVE tensor_scalar max / ACT Relu).
    Nodes in the output set write v directly into an interleaved output
    tile [p, f*64 + j] so the final DMA to the [n_samples, 64] output is
    fully contiguous per partition.
  - Only ancestors of the chosen output nodes are computed.
"""

import numpy as np
import ml_dtypes

N_CORES = 8
P = 128  # SBUF partitions
CAL_FRAC = 0.1
SIGMA_DROP = 1e-4  # noise rows with sigma below this are not uploaded


def _host_pilot_sigma(W_eff, b, parents, is_root, root_pilot):
    """Noiseless pilot scan + per-node sigma = CAL_FRAC * IQR (host, f32)."""
    n_nodes = len(parents)
    n = root_pilot.shape[1]
    vals = np.zeros((n_nodes, n), np.float32)
    for i in range(n_nodes):
        if is_root[i]:
            v = root_pilot[i].astype(np.float32)
        else:
            h = np.zeros(n, np.float32)
            for p, w in parents[i]:
                h = h + np.float32(w) * vals[p]
            v = np.maximum(h + np.float32(b[i]), np.float32(0.0))
        v = np.where(np.isfinite(v), v, np.float32(0.0))
        vals[i] = v
    q75 = np.quantile(vals.astype(np.float64), 0.75, axis=1)
    q25 = np.quantile(vals.astype(np.float64), 0.25, axis=1)
    sigma = CAL_FRAC * np.maximum(q75 - q25, 1e-6)
    return sigma.astype(np.float32)


def _dag_structure(W, b, par_idx, par_mask, is_root, chosen):
    n_nodes = W.shape[0]
    W_eff = (np.asarray(W, np.float32) * np.asarray(par_mask, np.float32))
    parents = []
    for i in range(n_nodes):
        ps = [
            (int(par_idx[i, j]), float(W_eff[i, j]))
            for j in range(par_idx.shape[1])
            if par_mask[i, j] > 0
        ]
        parents.append(ps)
    # needed = chosen + all ancestors
    needed = set(int(c) for c in chosen)
    for i in range(n_nodes - 1, -1, -1):
        if i in needed and not is_root[i]:
            for p, _ in parents[i]:
                needed.add(p)
    return W_eff, parents, needed


def _build_program(NLOC, parents, is_root, chosen, needed, b, sigma, n_nodes,
                   gz=13, repeats=1, plan="slack", slack_thresh=2,
                   act_frac=0.45, pool_frac=0.0):
    """Trace the per-core Bass/Tile program. Returns (nc, z_rows, root_rows)."""
    from concourse import bacc
    import concourse.mybir as mybir
    from concourse.tile import TileContext

    F = NLOC // P
    assert NLOC % P == 0

    f32 = mybir.dt.float32
    f16 = mybir.dt.float16
    f8 = mybir.dt.float8e4
    cdt = f16
    AF = mybir.ActivationFunctionType
    OP = mybir.AluOpType

    # columns of the output each node must write (normally 0 or 1)
    node_cols = {}
    for j, c in enumerate(int(c) for c in chosen):
        node_cols.setdefault(c, []).append(j)
    n_out = len(chosen)

    topo = [i for i in range(n_nodes) if i in needed]  # index order == topo
    topo_pos = {node: k for k, node in enumerate(topo)}
    # noisy rows: needed, non-root, sigma above drop threshold
    z_rows = [i for i in topo if not is_root[i] and sigma[i] >= SIGMA_DROP]
    z_row_of = {node: r for r, node in enumerate(z_rows)}
    root_rows = [i for i in topo if is_root[i]]
    root_row_of = {node: r for r, node in enumerate(root_rows)}
    # nodes that need a contiguous vals slice: non-chosen needed nodes,
    # plus all roots (DMA target must be contiguous)
    vals_nodes = [i for i in topo if i not in node_cols or is_root[i]]
    n_z = len(z_rows)

    nc = bacc.Bacc(None, target_bir_lowering=False)
    # z: partition-major fp8 [P, n_z*F]; cast to fp16 during load DMA
    z_in = nc.dram_tensor("zin", [P, max(n_z, 1) * F], f8,
                          kind="ExternalInput")
    root_in = nc.dram_tensor("rootin", [max(len(root_rows), 1), NLOC], f16,
                             kind="ExternalInput")
    out_d = nc.dram_tensor("out", [NLOC, n_out], f16, kind="ExternalOutput")

    with TileContext(nc) as tc:
        with tc.tile_pool(name="vals", bufs=1) as vpool, \
             tc.tile_pool(name="zpool", bufs=4) as zpool, \
             tc.tile_pool(name="tmp", bufs=24) as tpool, \
             tc.tile_pool(name="outp", bufs=1) as opool:

            out_t = opool.tile([P, n_out * F], cdt, tag="out", name="out_t")
            out_cols = out_t[:].rearrange("p (f j) -> p j f", j=n_out)

            vtile = {}
            for i in vals_nodes:
                vtile[i] = vpool.tile([P, F], cdt, tag=f"v{i}", name=f"vt{i}")

            def col_ap(j):
                return out_cols[:, j]

            def src_ap(node):
                if node in vtile:
                    return vtile[node][:]
                return col_ap(node_cols[node][0])

            def dst_aps(node):
                if node in node_cols and not is_root[node]:
                    return [col_ap(j) for j in node_cols[node]]
                return [vtile[node][:]]

            def trace_body(rep):
                # root rows: DMA fp16 straight into the vals slice
                for r in root_rows:
                    nc.sync.dma_start(
                        out=vtile[r][:],
                        in_=root_in[root_row_of[r]:root_row_of[r] + 1, :]
                            .rearrange("o (p f) -> (o p) f", p=P),
                    )
                    for j in node_cols.get(r, []):
                        nc.scalar.copy(out=col_ap(j), in_=vtile[r][:])

                # z row groups (packed order); fp8 -> fp16 cast during DMA
                z_group_tiles = {}

                def ensure_z_group(g):
                    if g in z_group_tiles:
                        return z_group_tiles[g]
                    r0 = g * gz
                    r1 = min(r0 + gz, n_z)
                    zt = zpool.tile([P, (r1 - r0) * F], cdt, tag="zg",
                                    name=f"zg{rep}_{g}")
                    nc.gpsimd.dma_start(
                        out=zt[:],
                        in_=z_in[:, r0 * F:r1 * F],
                    )
                    z_group_tiles[g] = zt
                    return zt

                def z_ap(node):
                    r = z_row_of[node]
                    g, k = divmod(r, gz)
                    zt = ensure_z_group(g)
                    return zt[:, k * F:(k + 1) * F]

                n_nonroot = max(len(z_rows), 1)
                for i in topo:
                    if is_root[i]:
                        continue
                    ps = parents[i]
                    bi = float(b[i])
                    noisy = i in z_row_of
                    dsts = dst_aps(i)
                    if len(ps) == 0:
                        # v = relu(b) + sigma*z in one op
                        c = max(bi, 0.0)
                        if noisy:
                            nc.vector.tensor_scalar(
                                out=dsts[0], in0=z_ap(i),
                                scalar1=1.0, scalar2=c, op0=OP.mult,
                                op1=OP.add)
                        else:
                            nc.vector.memset(dsts[0], c)
                    elif len(ps) == 1 and not noisy:
                        # v = relu(w*p + b): single ACT op
                        nc.scalar.activation(
                            dsts[0], src_ap(ps[0][0]), AF.Relu,
                            bias=bi, scale=ps[0][1])
                    else:
                        if len(ps) >= 2:
                            # pick the parent with more scheduling slack
                            # for op1 (off-chain candidate)
                            pa, pb = ps[1], ps[0]
                            if plan == "slack":
                                pa, pb = sorted(
                                    ps, key=lambda pw: topo_pos[pw[0]])[0:2]
                            u_t = tpool.tile([P, F], cdt, tag="u",
                                             name=f"u{rep}_{i}")
                            op1_act = (
                                plan == "base"
                                or (plan == "slack"
                                    and topo_pos[i] - topo_pos[pa[0]]
                                    >= slack_thresh and bi == 0.0))
                            frac = topo_pos[i] / max(len(topo) - 1, 1)
                            if op1_act and bi == 0.0 and frac < act_frac + 0.55:
                                nc.scalar.activation(
                                    u_t[:], src_ap(pa[0]), AF.Identity,
                                    bias=bi, scale=pa[1])
                            else:
                                nc.vector.tensor_scalar(
                                    out=u_t[:], in0=src_ap(pa[0]),
                                    scalar1=pa[1], scalar2=bi,
                                    op0=OP.mult, op1=OP.add)
                            s_t = tpool.tile([P, F], cdt, tag="s",
                                             name=f"s{rep}_{i}")
                            nc.vector.scalar_tensor_tensor(
                                out=s_t[:], in0=src_ap(pb[0]),
                                scalar=pb[1], in1=u_t[:],
                                op0=OP.mult, op1=OP.add)
                        else:
                            s_t = tpool.tile([P, F], cdt, tag="s",
                                             name=f"s{rep}_{i}")
                            nc.vector.tensor_scalar(
                                out=s_t[:], in0=src_ap(ps[0][0]),
                                scalar1=ps[0][1], scalar2=bi,
                                op0=OP.mult, op1=OP.add)
                        if noisy:
                            # v = max(s, 0) + zs  (fused relu + noise)
                            nc.vector.scalar_tensor_tensor(
                                out=dsts[0], in0=s_t[:], scalar=0.0,
                                in1=z_ap(i), op0=OP.max, op1=OP.add)
                        else:
                            # v = max(s, 0)
                            nc.vector.tensor_scalar(
                                out=dsts[0], in0=s_t[:],
                                scalar1=0.0, scalar2=None, op0=OP.max)
                    for extra in dsts[1:]:
                        nc.scalar.copy(out=extra, in_=dsts[0])

                # output DMA: all 128 partitions per transfer, split along
                # the free dim across several dma_starts
                out_ap = out_d[:, :].rearrange("(p f) j -> p (f j)", p=P)
                FSPLIT = (F + 3) // 4
                for f0 in range(0, F, FSPLIT):
                    f1 = min(f0 + FSPLIT, F)
                    nc.sync.dma_start(
                        out=out_ap[:, f0 * n_out:f1 * n_out],
                        in_=out_t[:, f0 * n_out:f1 * n_out])

            for rep in range(repeats):
                trace_body(rep)

    nc.finalize()
    return nc, z_rows, root_rows


_CACHE = {}
_LAST_NC = None
_LAST_IN_MAPS = None


def _get_program(key, *args, **kwargs):
    if key not in _CACHE:
        _CACHE[key] = _build_program(*args, **kwargs)
    return _CACHE[key]


def run(n_samples, W, b, root_pilot, root_main, z_noise, par_mask, par_idx,
        is_root, chosen, trace=False, n_cores=N_CORES, gz=13,
        repeats=1, plan="slack", slack_thresh=2, act_frac=0.45,
        pool_frac=0.0):
    W = np.asarray(W, np.float32)
    b = np.asarray(b, np.float32)
    root_pilot = np.asarray(root_pilot, np.float32)
    root_main = np.asarray(root_main, np.float32)
    z_noise = np.asarray(z_noise, np.float32)
    par_mask = np.asarray(par_mask, np.float32)
    par_idx = np.asarray(par_idx, np.int32)
    is_root = np.asarray(is_root, bool)
    chosen = np.asarray(chosen, np.int32)

    n_nodes = W.shape[0]
    NS = root_main.shape[1]
    assert NS % (n_cores * P) == 0
    NLOC = NS // n_cores
    F = NLOC // P

    W_eff, parents, needed = _dag_structure(W, b, par_idx, par_mask, is_root,
                                            chosen)
    sigma = _host_pilot_sigma(W_eff, b, parents, is_root, root_pilot)

    key = (NLOC, n_nodes, tuple(chosen.tolist()), par_idx.tobytes(),
           par_mask.tobytes(), W_eff.tobytes(), b.tobytes(), sigma.tobytes(),
           is_root.tobytes(), gz, repeats, plan, slack_thresh, act_frac,
           pool_frac)
    nc, z_rows, root_rows = _get_program(
        key, NLOC, parents, is_root, chosen, needed, b, sigma, n_nodes,
        gz=gz, repeats=repeats, plan=plan, slack_thresh=slack_thresh,
        act_frac=act_frac, pool_frac=pool_frac)

    n_z = len(z_rows)
    if n_z:
        zsel = z_noise[z_rows] * sigma[z_rows][:, None]
        # [n_z, NS] -> per-core partition-major [P, n_z*F] fp8
        z8 = zsel.reshape(n_z, n_cores, P, F).astype(ml_dtypes.float8_e4m3)
        z8 = z8.transpose(1, 2, 0, 3).reshape(n_cores, P, n_z * F)
    else:
        z8 = np.zeros((n_cores, P, F), ml_dtypes.float8_e4m3)
    root_packed = (root_main[root_rows].astype(np.float16) if root_rows
                   else np.zeros((1, NS), np.float16))

    in_maps = []
    for c in range(n_cores):
        s0, s1 = c * NLOC, (c + 1) * NLOC
        in_maps.append({
            "zin": np.ascontiguousarray(z8[c]),
            "rootin": np.ascontiguousarray(root_packed[:, s0:s1]),
        })

    from concourse.bass_utils import run_bass_kernel_spmd
    global _LAST_NC, _LAST_IN_MAPS
    _LAST_NC, _LAST_IN_MAPS = nc, in_maps
    res = run_bass_kernel_spmd(nc, in_maps, core_ids=list(range(n_cores)),
                               trace=trace)
    out = np.concatenate([np.asarray(r["out"]) for r in res.results], axis=0)
    return out.astype(np.float32), res


def kernel(**inputs):
    out, _ = run(**inputs)
    return out
